# revision 15
# baseline (speedup 1.0000x reference)
"""Trainium2 Bass kernel for an 11-stage butterfly linear layer + bias.

Problem: x (16384, 2048) fp32; out[b, :] = B @ x[b, :] + bias where B is the
composition of 11 butterfly stages (strides 1..1024), each an elementwise 2x2
mix of position pairs with learned per-pair coefficients.

Factorization used here (positions p = blk*128 + w, blk in [0,16), w in [0,128)):
  - Stages 0-6 (strides 1..64) mix only within a 128-block -> block-diagonal
    D = diag(D_0..D_15), each 128x128 dense.
  - Stages 7-10 (strides 128..1024) mix across blocks, separately per w ->
    per-w 16x16 matrices C_w.  Regrouping positions as q = blk*8 + (w mod 8)
    within w-group t = w//8 makes this another block-diagonal transform
    C't = diag over the 8 w's in group t of C_w (128x128 per group).

Device pipeline per 256-row batch chunk (per core; batch is sharded 8 ways):
  DMA in -> TensorE transpose (position onto partitions) -> 16 matmuls (D)
  -> SBUF->SBUF DMA partition regroup -> 16 matmuls (C') with bias folded into
  the per-partition ScalarE PSUM drain -> TensorE transpose back -> DMA out.
"""

import sys

import numpy as np

sys.path.insert(0, "/opt/trn_rl_repo")

import concourse.bass as bass  # noqa: E402
import concourse.mybir as mybir  # noqa: E402
import concourse.tile as tile  # noqa: E402
from concourse import bacc  # noqa: E402
from concourse.bass import ds, ts  # noqa: E402
from concourse.bass_utils import run_bass_kernel_spmd  # noqa: E402

N = 2048
LOG_N = 11
NCORES = 8
BATCH = 16384
BPC = BATCH // NCORES  # batch rows per core
P = 128
NB = 16  # number of 128-blocks
CH = 512  # batch rows per pipeline chunk

# fp32r streams the PE at 1 col/cycle for moving dim >= 256 (plain fp32 is
# 4 cycles/col); numerically it is fp32 in/out.
MM_F32R = True
TR_F32R = False  # transpose streaming dtype (fp32: 2 c/r, f32r: 1.5 c/r)
MID_BF16 = False  # carry the mid tensor (pass-1 output) + pass-2 matmuls in bf16

REPEAT = 1  # whole-pipeline repetitions (for on-device timing via run-length diff)

PROFILE = False
LAST_RESULTS = None

_NC_CACHE = {}


def _emit_body(ctx, tc, aps, bpc):
    nc = tc.nc
    x_ap, w1_ap, c2_ap, bt_ap, bb_ap, id_ap, out_ap = aps
    f32 = mybir.dt.float32
    mmdt = mybir.dt.float32r if MM_F32R else f32
    middt = mybir.dt.bfloat16 if MID_BF16 else mmdt
    trdt = mybir.dt.float32r if TR_F32R else f32
    nch = bpc // CH

    const = ctx.enter_context(tc.tile_pool(name="const", bufs=1))
    W1 = const.tile([P, NB * P], mmdt)
    C2 = const.tile([P, NB * P], middt)
    BT = const.tile([P, NB], f32)
    BB = const.tile([P, N], f32)
    ID = const.tile([P, P], f32)
    scratch = ctx.enter_context(tc.tile_pool(name="cscratch", bufs=1))
    W1f = scratch.tile([P, NB * P], f32, name="W1f", tag="cs")
    nc.sync.dma_start(W1f[:], w1_ap)
    nc.scalar.copy(W1[:], W1f[:])
    C2f = scratch.tile([P, NB * P], f32, name="C2f", tag="cs")
    nc.sync.dma_start(C2f[:], c2_ap)
    nc.scalar.copy(C2[:], C2f[:])
    nc.sync.dma_start(BT[:], bt_ap)
    nc.sync.dma_start(BB[:], bb_ap)
    nc.sync.dma_start(ID[:], id_ap)

    xpool = ctx.enter_context(tc.tile_pool(name="xin", bufs=5 if MID_BF16 else 2))
    apool = ctx.enter_context(tc.tile_pool(name="amid", bufs=1))
    ypool = ctx.enter_context(tc.tile_pool(name="ymid", bufs=2 if MID_BF16 else 1))
    yppool = ctx.enter_context(tc.tile_pool(name="ypmid", bufs=2 if MID_BF16 else 1))
    opool = ctx.enter_context(tc.tile_pool(name="oout", bufs=5 if MID_BF16 else 2))
    if MID_BF16:
        zpool = None
        ps_to = None
        ps_ti = ctx.enter_context(tc.tile_pool(name="ps_ti", bufs=3, space="PSUM"))
        ps_m1 = ctx.enter_context(tc.tile_pool(name="ps_m1", bufs=2, space="PSUM"))
        ps_m2 = ctx.enter_context(tc.tile_pool(name="ps_m2", bufs=3, space="PSUM"))
    else:
        zpool = ctx.enter_context(tc.tile_pool(name="zmid", bufs=1))
        ps_ti = ctx.enter_context(tc.tile_pool(name="ps_ti", bufs=2, space="PSUM"))
        ps_to = ctx.enter_context(tc.tile_pool(name="ps_to", bufs=2, space="PSUM"))
        ps_m1 = ctx.enter_context(tc.tile_pool(name="ps_m1", bufs=2, space="PSUM"))
        ps_m2 = ctx.enter_context(tc.tile_pool(name="ps_m2", bufs=2, space="PSUM"))

    for rep_ci in range(REPEAT * nch):
        rep, ci = divmod(rep_ci, nch)
        r0 = ci * CH
        HQ = CH // P  # 128-row quarters per chunk
        xts = []
        for h in range(HQ):
            xt = xpool.tile([P, N], f32, name=f"xt_{rep_ci}_{h}", tag="xt")
            nc.sync.dma_start(xt[:], x_ap[r0 + h * P : r0 + (h + 1) * P, :])
            xts.append(xt)

        # --- T_in: A[w, b*CH + h*P + f] = x[r0 + h*P + f, b*P + w] ---
        A = apool.tile([P, NB * CH], mmdt, name=f"A_{rep_ci}", tag="A")
        for h in range(HQ):
            for bq in range(4):
                tp = ps_ti.tile([P, 4 * P], f32, name=f"tp_{rep_ci}_{h}_{bq}", tag="tp")
                for j in range(4):
                    b = bq * 4 + j
                    nc.tensor.transpose(
                        tp[:, ts(j, P)].bitcast(trdt),
                        xts[h][:, ts(b, P)].bitcast(trdt),
                        ID[:].bitcast(trdt),
                    )
                dst = A[:].rearrange("p (b hh f) -> p b hh f", b=NB, hh=HQ, f=P)[
                    :, bq * 4 : (bq + 1) * 4, h
                ]
                src = tp[:].rearrange("p (j f) -> p j f", j=4, f=P)
                nc.scalar.copy(dst, src)

        # --- MM1: Y_b = D_b-transform, partitions = within-block out pos ---
        Ysb = ypool.tile([P, NB * CH], middt, name=f"Y_{rep_ci}", tag="Y")
        for b in range(NB):
            py = ps_m1.tile([P, CH], f32, name=f"py_{rep_ci}_{b}", tag="py")
            nc.tensor.matmul(
                py[:],
                W1[:, ts(b, P)],
                A[:, ts(b, CH)],
                start=True,
                stop=True,
            )
            nc.scalar.copy(Ysb[:, ts(b, CH)], py[:])

        # --- permute: Yp[w8*16 + b, t*CH + f] = Ysb[t*8 + w8, b*CH + f] ---
        # (dst is a plain 2D slice; src crosses partitions only in dim0)
        Yp = yppool.tile([P, NB * CH], middt, name=f"Yp_{rep_ci}", tag="Yp")
        for t in range(NB):
            psrc = Ysb[8 * t : 8 * t + 8, :].rearrange("w (b f) -> w b f", b=16, f=CH)
            eng = nc.sync if t % 2 == 0 else nc.scalar
            eng.dma_start(Yp[:, ts(t, CH)], psrc)

        if MID_BF16:
            # --- MM2' (data stationary): out = Yp_slice.T @ C2_t, natural
            # batch-major output; bias added in the strided PSUM drain ---
            Os = []
            for h in range(HQ):
                O = opool.tile([P, N], f32, name=f"O_{rep_ci}_{h}", tag="O")
                Os.append(O)
            for t in range(NB):
                pz = ps_m2.tile([P, HQ * P], f32, name=f"pz_{rep_ci}_{t}", tag="pz")
                for h in range(HQ):
                    nc.tensor.matmul(
                        pz[:, ts(h, P)],
                        Yp[:, ds(t * CH + h * P, P)],
                        C2[:, ts(t, P)],
                        start=True,
                        stop=True,
                    )
                for h in range(HQ):
                    dst = Os[h][:].rearrange("p (b t w) -> p t b w", b=16, t=16, w=8)[:, t]
                    psrc = pz[:, ts(h, P)].rearrange("p (b w) -> p b w", b=16, w=8)
                    bsrc = BB[:].rearrange("p (b t w) -> p t b w", b=16, t=16, w=8)[:, t]
                    nc.vector.tensor_add(dst, psrc, bsrc)
            for h in range(HQ):
                nc.sync.dma_start(out_ap[r0 + h * P : r0 + (h + 1) * P, :], Os[h][:])
        else:
            # --- MM2 (+bias): Z_t = C't-transform, partitions n = b*8 + w8 ---
            Zsb = zpool.tile([P, NB * CH], f32, name=f"Z_{rep_ci}", tag="Z")
            for t in range(NB):
                pz = ps_m2.tile([P, CH], f32, name=f"pz_{rep_ci}_{t}", tag="pz")
                nc.tensor.matmul(
                    pz[:],
                    C2[:, ts(t, P)],
                    Yp[:, ts(t, CH)],
                    start=True,
                    stop=True,
                )
                nc.scalar.add(Zsb[:, ts(t, CH)], pz[:], BT[:, ts(t, 1)])

            # --- T_out + scatter-drain + DMA out ---
            for h in range(HQ):
                O = opool.tile([P, N], f32, name=f"O_{rep_ci}_{h}", tag="O")
                for tq in range(4):
                    po = ps_to.tile([P, 4 * P], f32, name=f"po_{rep_ci}_{h}_{tq}", tag="po")
                    for j in range(4):
                        t = tq * 4 + j
                        nc.tensor.transpose(
                            po[:, ts(j, P)].bitcast(trdt),
                            Zsb[:, ds(t * CH + h * P, P)].bitcast(trdt),
                            ID[:].bitcast(trdt),
                        )
                    for j in range(4):
                        t = tq * 4 + j
                        dst = O[:].rearrange("p (b t w) -> p t b w", b=16, t=16, w=8)[:, t]
                        src = po[:, ts(j, P)].rearrange("p (b w) -> p b w", b=16, w=8)
                        nc.vector.tensor_copy(dst, src)
                nc.sync.dma_start(out_ap[r0 + h * P : r0 + (h + 1) * P, :], O[:])


def build_nc(bpc=BPC):
    nc = bacc.Bacc(
        "TRN2",
        target_bir_lowering=False,
        debug=False,
        num_devices=NCORES,
    )
    x_ap = nc.dram_tensor("x", [bpc, N], mybir.dt.float32, kind="ExternalInput").ap()
    w1_ap = nc.dram_tensor(
        "w1", [P, NB * P], mybir.dt.float32, kind="ExternalInput"
    ).ap()
    c2_ap = nc.dram_tensor(
        "c2", [P, NB * P], mybir.dt.float32, kind="ExternalInput"
    ).ap()
    bt_ap = nc.dram_tensor("bt", [P, NB], mybir.dt.float32, kind="ExternalInput").ap()
    bb_ap = nc.dram_tensor("bb", [P, N], mybir.dt.float32, kind="ExternalInput").ap()
    id_ap = nc.dram_tensor("ident", [P, P], mybir.dt.float32, kind="ExternalInput").ap()
    out_ap = nc.dram_tensor("out", [bpc, N], mybir.dt.float32, kind="ExternalOutput").ap()

    from contextlib import ExitStack

    with tile.TileContext(nc) as tc:
        with ExitStack() as ctx:
            _emit_body(ctx, tc, (x_ap, w1_ap, c2_ap, bt_ap, bb_ap, id_ap, out_ap), bpc)
    nc.compile()
    return nc


def _butterfly_apply(tw, X, idx_lo, idx_hi):
    """Apply butterfly stages [idx_lo, idx_hi) to rows of X. tw: (LOG_N, N//2, 2, 2)."""
    out = X
    for idx in range(idx_lo, idx_hi):
        s = 1 << idx
        g = N // (2 * s)
        T = tw[idx].reshape(g, s, 2, 2)
        xr = out.reshape(-1, g, 2, s)
        out = np.einsum("gsij,bgjs->bgis", T, xr).reshape(-1, N)
    return out


def host_weights(twiddle, bias):
    """Build device constants from the twiddle/bias arrays."""
    tw = np.asarray(twiddle, dtype=np.float64)[0, 0]  # (LOG_N, N//2, 2, 2)
    eye = np.eye(N, dtype=np.float64)
    R1 = _butterfly_apply(tw, eye, 0, 7)  # = D^T, block-diagonal
    R2 = _butterfly_apply(tw, eye, 7, LOG_N)  # = C^T

    # W1 lhsT per block b: lhsT[k, m] = D_b[m, k] = R1 block (b, b)
    w1 = np.concatenate(
        [R1[b * P : (b + 1) * P, b * P : (b + 1) * P] for b in range(NB)], axis=1
    )
    # C2 lhsT per w-group t: rows q = w8*16+b2 (mid pos), cols n = b*8+w8 (out pos)
    c2 = np.zeros((P, NB * P))
    q = np.arange(P)
    for t in range(NB):
        pm = (q % 16) * P + t * 8 + (q // 16)  # row order: q = w8*16 + b2
        pn = (q // 8) * P + t * 8 + (q % 8)  # col order: n = b*8 + w8
        c2[:, t * P : (t + 1) * P] = R2[np.ix_(pm, pn)]
    # bias per partition n for group t: bias[(n//8)*128 + t*8 + n%8]
    bt = np.zeros((P, NB))
    b64 = np.asarray(bias, dtype=np.float64)
    for t in range(NB):
        bt[:, t] = b64[(q // 8) * P + t * 8 + (q % 8)]
    bb = np.broadcast_to(b64[None, :], (P, N))
    ident = np.eye(P)
    return (
        np.ascontiguousarray(w1, dtype=np.float32),
        np.ascontiguousarray(c2, dtype=np.float32),
        np.ascontiguousarray(bt, dtype=np.float32),
        np.ascontiguousarray(bb, dtype=np.float32),
        np.ascontiguousarray(ident, dtype=np.float32),
    )


def kernel(x, twiddle, bias):
    global LAST_RESULTS
    x = np.ascontiguousarray(np.asarray(x), dtype=np.float32)
    assert x.shape == (BATCH, N), x.shape

    key = (BPC, REPEAT)
    if key not in _NC_CACHE:
        _NC_CACHE[key] = build_nc(BPC)
    nc = _NC_CACHE[key]

    w1, c2, bt, bb, ident = host_weights(twiddle, bias)
    in_maps = [
        {
            "x": x[c * BPC : (c + 1) * BPC],
            "w1": w1,
            "c2": c2,
            "bt": bt,
            "bb": bb,
            "ident": ident,
        }
        for c in range(NCORES)
    ]
    res = run_bass_kernel_spmd(
        nc, in_maps, core_ids=list(range(NCORES)), trace=PROFILE
    )
    LAST_RESULTS = res
    out = np.concatenate([res.results[c]["out"] for c in range(NCORES)], axis=0)
    return out


# revision 16
# speedup vs baseline: 1.5877x; 1.5877x over previous
"""Trainium2 Bass kernel for an 11-stage butterfly linear layer + bias.

Problem: x (16384, 2048) fp32; out[b, :] = B @ x[b, :] + bias where B is the
composition of 11 butterfly stages (strides 1..1024), each an elementwise 2x2
mix of position pairs with learned per-pair coefficients.

Factorization used here (positions p = blk*128 + w, blk in [0,16), w in [0,128)):
  - Stages 0-6 (strides 1..64) mix only within a 128-block -> block-diagonal
    D = diag(D_0..D_15), each 128x128 dense.
  - Stages 7-10 (strides 128..1024) mix across blocks, separately per w ->
    per-w 16x16 matrices C_w.  Regrouping positions as q = blk*8 + (w mod 8)
    within w-group t = w//8 makes this another block-diagonal transform
    C't = diag over the 8 w's in group t of C_w (128x128 per group).

Device pipeline per 256-row batch chunk (per core; batch is sharded 8 ways):
  DMA in -> TensorE transpose (position onto partitions) -> 16 matmuls (D)
  -> SBUF->SBUF DMA partition regroup -> 16 matmuls (C') with bias folded into
  the per-partition ScalarE PSUM drain -> TensorE transpose back -> DMA out.
"""

import sys

import numpy as np

sys.path.insert(0, "/opt/trn_rl_repo")

import concourse.bass as bass  # noqa: E402
import concourse.mybir as mybir  # noqa: E402
import concourse.tile as tile  # noqa: E402
from concourse import bacc  # noqa: E402
from concourse.bass import ds, ts  # noqa: E402
from concourse.bass_utils import run_bass_kernel_spmd  # noqa: E402

N = 2048
LOG_N = 11
NCORES = 8
BATCH = 16384
BPC = BATCH // NCORES  # batch rows per core
P = 128
NB = 16  # number of 128-blocks
CH = 512  # batch rows per pipeline chunk

# fp32r streams the PE at 1 col/cycle for moving dim >= 256 (plain fp32 is
# 4 cycles/col); numerically it is fp32 in/out.
MM_F32R = True
TR_F32R = False  # transpose streaming dtype (fp32: 2 c/r, f32r: 1.5 c/r)
MID_BF16 = True  # carry the mid tensor (pass-1 output) + pass-2 matmuls in bf16

REPEAT = 1  # whole-pipeline repetitions (for on-device timing via run-length diff)

PROFILE = False
LAST_RESULTS = None

_NC_CACHE = {}


def _emit_body(ctx, tc, aps, bpc):
    nc = tc.nc
    x_ap, w1_ap, c2_ap, bt_ap, bb_ap, id_ap, out_ap = aps
    f32 = mybir.dt.float32
    mmdt = mybir.dt.float32r if MM_F32R else f32
    middt = mybir.dt.bfloat16 if MID_BF16 else mmdt
    trdt = mybir.dt.float32r if TR_F32R else f32
    nch = bpc // CH

    const = ctx.enter_context(tc.tile_pool(name="const", bufs=1))
    W1 = const.tile([P, NB * P], mmdt)
    C2 = const.tile([P, NB * P], middt)
    BT = const.tile([P, NB], f32)
    BB = const.tile([P, N], f32)
    ID = const.tile([P, P], f32)
    scratch = ctx.enter_context(tc.tile_pool(name="cscratch", bufs=1))
    W1f = scratch.tile([P, NB * P], f32, name="W1f", tag="cs")
    nc.sync.dma_start(W1f[:], w1_ap)
    nc.scalar.copy(W1[:], W1f[:])
    C2f = scratch.tile([P, NB * P], f32, name="C2f", tag="cs")
    nc.sync.dma_start(C2f[:], c2_ap)
    nc.scalar.copy(C2[:], C2f[:])
    nc.sync.dma_start(BT[:], bt_ap)
    nc.sync.dma_start(BB[:], bb_ap)
    nc.sync.dma_start(ID[:], id_ap)

    xpool = ctx.enter_context(tc.tile_pool(name="xin", bufs=5 if MID_BF16 else 2))
    apool = ctx.enter_context(tc.tile_pool(name="amid", bufs=1))
    ypool = ctx.enter_context(tc.tile_pool(name="ymid", bufs=2 if MID_BF16 else 1))
    yppool = ctx.enter_context(tc.tile_pool(name="ypmid", bufs=2 if MID_BF16 else 1))
    opool = ctx.enter_context(tc.tile_pool(name="oout", bufs=5 if MID_BF16 else 2))
    if MID_BF16:
        zpool = None
        ps_to = None
        ps_ti = ctx.enter_context(tc.tile_pool(name="ps_ti", bufs=3, space="PSUM"))
        ps_m1 = ctx.enter_context(tc.tile_pool(name="ps_m1", bufs=2, space="PSUM"))
        ps_m2 = ctx.enter_context(tc.tile_pool(name="ps_m2", bufs=3, space="PSUM"))
    else:
        zpool = ctx.enter_context(tc.tile_pool(name="zmid", bufs=1))
        ps_ti = ctx.enter_context(tc.tile_pool(name="ps_ti", bufs=2, space="PSUM"))
        ps_to = ctx.enter_context(tc.tile_pool(name="ps_to", bufs=2, space="PSUM"))
        ps_m1 = ctx.enter_context(tc.tile_pool(name="ps_m1", bufs=2, space="PSUM"))
        ps_m2 = ctx.enter_context(tc.tile_pool(name="ps_m2", bufs=2, space="PSUM"))

    for rep_ci in range(REPEAT * nch):
        rep, ci = divmod(rep_ci, nch)
        r0 = ci * CH
        HQ = CH // P  # 128-row quarters per chunk
        xts = []
        for h in range(HQ):
            xt = xpool.tile([P, N], f32, name=f"xt_{rep_ci}_{h}", tag="xt")
            nc.sync.dma_start(xt[:], x_ap[r0 + h * P : r0 + (h + 1) * P, :])
            xts.append(xt)

        # --- T_in: A[w, b*CH + h*P + f] = x[r0 + h*P + f, b*P + w] ---
        A = apool.tile([P, NB * CH], mmdt, name=f"A_{rep_ci}", tag="A")
        for h in range(HQ):
            for bq in range(4):
                tp = ps_ti.tile([P, 4 * P], f32, name=f"tp_{rep_ci}_{h}_{bq}", tag="tp")
                for j in range(4):
                    b = bq * 4 + j
                    nc.tensor.transpose(
                        tp[:, ts(j, P)].bitcast(trdt),
                        xts[h][:, ts(b, P)].bitcast(trdt),
                        ID[:].bitcast(trdt),
                    )
                dst = A[:].rearrange("p (b hh f) -> p b hh f", b=NB, hh=HQ, f=P)[
                    :, bq * 4 : (bq + 1) * 4, h
                ]
                src = tp[:].rearrange("p (j f) -> p j f", j=4, f=P)
                nc.scalar.copy(dst, src)

        # --- MM1: Y_b = D_b-transform, partitions = within-block out pos ---
        Ysb = ypool.tile([P, NB * CH], middt, name=f"Y_{rep_ci}", tag="Y")
        for b in range(NB):
            py = ps_m1.tile([P, CH], f32, name=f"py_{rep_ci}_{b}", tag="py")
            nc.tensor.matmul(
                py[:],
                W1[:, ts(b, P)],
                A[:, ts(b, CH)],
                start=True,
                stop=True,
            )
            nc.scalar.copy(Ysb[:, ts(b, CH)], py[:])

        # --- permute: Yp[w8*16 + b, t*CH + f] = Ysb[t*8 + w8, b*CH + f] ---
        # (dst is a plain 2D slice; src crosses partitions only in dim0)
        Yp = yppool.tile([P, NB * CH], middt, name=f"Yp_{rep_ci}", tag="Yp")
        for t in range(NB):
            psrc = Ysb[8 * t : 8 * t + 8, :].rearrange("w (b f) -> w b f", b=16, f=CH)
            eng = nc.sync if t % 2 == 0 else nc.scalar
            eng.dma_start(Yp[:, ts(t, CH)], psrc)

        if MID_BF16:
            # --- MM2' (data stationary): out = Yp_slice.T @ C2_t, natural
            # batch-major output; bias added in the strided PSUM drain ---
            Os = []
            for h in range(HQ):
                O = opool.tile([P, N], f32, name=f"O_{rep_ci}_{h}", tag="O")
                Os.append(O)
            for t in range(NB):
                pz = ps_m2.tile([P, HQ * P], f32, name=f"pz_{rep_ci}_{t}", tag="pz")
                for h in range(HQ):
                    nc.tensor.matmul(
                        pz[:, ts(h, P)],
                        Yp[:, ds(t * CH + h * P, P)],
                        C2[:, ts(t, P)],
                        start=True,
                        stop=True,
                    )
                for h in range(HQ):
                    dst = Os[h][:].rearrange("p (b t w) -> p t b w", b=16, t=16, w=8)[:, t]
                    psrc = pz[:, ts(h, P)].rearrange("p (b w) -> p b w", b=16, w=8)
                    bsrc = BB[:].rearrange("p (b t w) -> p t b w", b=16, t=16, w=8)[:, t]
                    nc.vector.tensor_add(dst, psrc, bsrc)
            for h in range(HQ):
                nc.sync.dma_start(out_ap[r0 + h * P : r0 + (h + 1) * P, :], Os[h][:])
        else:
            # --- MM2 (+bias): Z_t = C't-transform, partitions n = b*8 + w8 ---
            Zsb = zpool.tile([P, NB * CH], f32, name=f"Z_{rep_ci}", tag="Z")
            for t in range(NB):
                pz = ps_m2.tile([P, CH], f32, name=f"pz_{rep_ci}_{t}", tag="pz")
                nc.tensor.matmul(
                    pz[:],
                    C2[:, ts(t, P)],
                    Yp[:, ts(t, CH)],
                    start=True,
                    stop=True,
                )
                nc.scalar.add(Zsb[:, ts(t, CH)], pz[:], BT[:, ts(t, 1)])

            # --- T_out + scatter-drain + DMA out ---
            for h in range(HQ):
                O = opool.tile([P, N], f32, name=f"O_{rep_ci}_{h}", tag="O")
                for tq in range(4):
                    po = ps_to.tile([P, 4 * P], f32, name=f"po_{rep_ci}_{h}_{tq}", tag="po")
                    for j in range(4):
                        t = tq * 4 + j
                        nc.tensor.transpose(
                            po[:, ts(j, P)].bitcast(trdt),
                            Zsb[:, ds(t * CH + h * P, P)].bitcast(trdt),
                            ID[:].bitcast(trdt),
                        )
                    for j in range(4):
                        t = tq * 4 + j
                        dst = O[:].rearrange("p (b t w) -> p t b w", b=16, t=16, w=8)[:, t]
                        src = po[:, ts(j, P)].rearrange("p (b w) -> p b w", b=16, w=8)
                        nc.vector.tensor_copy(dst, src)
                nc.sync.dma_start(out_ap[r0 + h * P : r0 + (h + 1) * P, :], O[:])


def build_nc(bpc=BPC):
    nc = bacc.Bacc(
        "TRN2",
        target_bir_lowering=False,
        debug=False,
        num_devices=NCORES,
    )
    x_ap = nc.dram_tensor("x", [bpc, N], mybir.dt.float32, kind="ExternalInput").ap()
    w1_ap = nc.dram_tensor(
        "w1", [P, NB * P], mybir.dt.float32, kind="ExternalInput"
    ).ap()
    c2_ap = nc.dram_tensor(
        "c2", [P, NB * P], mybir.dt.float32, kind="ExternalInput"
    ).ap()
    bt_ap = nc.dram_tensor("bt", [P, NB], mybir.dt.float32, kind="ExternalInput").ap()
    bb_ap = nc.dram_tensor("bb", [P, N], mybir.dt.float32, kind="ExternalInput").ap()
    id_ap = nc.dram_tensor("ident", [P, P], mybir.dt.float32, kind="ExternalInput").ap()
    out_ap = nc.dram_tensor("out", [bpc, N], mybir.dt.float32, kind="ExternalOutput").ap()

    from contextlib import ExitStack

    with tile.TileContext(nc) as tc:
        with ExitStack() as ctx:
            _emit_body(ctx, tc, (x_ap, w1_ap, c2_ap, bt_ap, bb_ap, id_ap, out_ap), bpc)
    nc.compile()
    return nc


def _butterfly_apply(tw, X, idx_lo, idx_hi):
    """Apply butterfly stages [idx_lo, idx_hi) to rows of X. tw: (LOG_N, N//2, 2, 2)."""
    out = X
    for idx in range(idx_lo, idx_hi):
        s = 1 << idx
        g = N // (2 * s)
        T = tw[idx].reshape(g, s, 2, 2)
        xr = out.reshape(-1, g, 2, s)
        out = np.einsum("gsij,bgjs->bgis", T, xr).reshape(-1, N)
    return out


def host_weights(twiddle, bias):
    """Build device constants from the twiddle/bias arrays."""
    tw = np.asarray(twiddle, dtype=np.float64)[0, 0]  # (LOG_N, N//2, 2, 2)
    eye = np.eye(N, dtype=np.float64)
    R1 = _butterfly_apply(tw, eye, 0, 7)  # = D^T, block-diagonal
    R2 = _butterfly_apply(tw, eye, 7, LOG_N)  # = C^T

    # W1 lhsT per block b: lhsT[k, m] = D_b[m, k] = R1 block (b, b)
    w1 = np.concatenate(
        [R1[b * P : (b + 1) * P, b * P : (b + 1) * P] for b in range(NB)], axis=1
    )
    # C2 lhsT per w-group t: rows q = w8*16+b2 (mid pos), cols n = b*8+w8 (out pos)
    c2 = np.zeros((P, NB * P))
    q = np.arange(P)
    for t in range(NB):
        pm = (q % 16) * P + t * 8 + (q // 16)  # row order: q = w8*16 + b2
        pn = (q // 8) * P + t * 8 + (q % 8)  # col order: n = b*8 + w8
        c2[:, t * P : (t + 1) * P] = R2[np.ix_(pm, pn)]
    # bias per partition n for group t: bias[(n//8)*128 + t*8 + n%8]
    bt = np.zeros((P, NB))
    b64 = np.asarray(bias, dtype=np.float64)
    for t in range(NB):
        bt[:, t] = b64[(q // 8) * P + t * 8 + (q % 8)]
    bb = np.broadcast_to(b64[None, :], (P, N))
    ident = np.eye(P)
    return (
        np.ascontiguousarray(w1, dtype=np.float32),
        np.ascontiguousarray(c2, dtype=np.float32),
        np.ascontiguousarray(bt, dtype=np.float32),
        np.ascontiguousarray(bb, dtype=np.float32),
        np.ascontiguousarray(ident, dtype=np.float32),
    )


def kernel(x, twiddle, bias):
    global LAST_RESULTS
    x = np.ascontiguousarray(np.asarray(x), dtype=np.float32)
    assert x.shape == (BATCH, N), x.shape

    key = (BPC, REPEAT)
    if key not in _NC_CACHE:
        _NC_CACHE[key] = build_nc(BPC)
    nc = _NC_CACHE[key]

    w1, c2, bt, bb, ident = host_weights(twiddle, bias)
    in_maps = [
        {
            "x": x[c * BPC : (c + 1) * BPC],
            "w1": w1,
            "c2": c2,
            "bt": bt,
            "bb": bb,
            "ident": ident,
        }
        for c in range(NCORES)
    ]
    res = run_bass_kernel_spmd(
        nc, in_maps, core_ids=list(range(NCORES)), trace=PROFILE
    )
    LAST_RESULTS = res
    out = np.concatenate([res.results[c]["out"] for c in range(NCORES)], axis=0)
    return out
